# revision 6
# baseline (speedup 1.0000x reference)
"""3D Gaussian blur (kernel_size=5, sigma=1.0) on (2,1,192,256,256) f32,
distributed over 8 Trainium2 NeuronCores.

The reference kernel factors exactly: g[i,j,l] = aD[i] * (1/5) * bW[l],
so the 3D conv separates into: Gaussian along D, box along H, Gaussian
along W.

Sharding: data-parallel over (batch, D-slab): 8 cores = 2 batches x 4
slabs of 48 output slices each; each core receives its slab plus a
2-slice halo (zero slices at batch edges), i.e. input [52, 256, 256].

Per-core kernel (Bass/Tile), v2 — D-conv via symmetric pairing:
  pairs (DVE + GpSimd): for each output slice dd, t1 = x[dd+1]+x[dd+3],
    t2 = x[dd]+x[dd+4] (bf16 adds batched 2 output slices per op on
    the contiguous flat x tile; every other t2 on GpSimd).
  pass A (PE, 12 banded matmuls/slice): y[dd] = sum over the 3 sources
    {x[dd+2], t1, t2} of (aD_v * box_H)-band matmuls, fusing the D
    Gaussian + H box conv with an H<->W transpose, PSUM-accumulated.
  evac A (ACT): PSUM f32 -> SBUF bf16 per 2 slices.
  pass B (PE, 4 banded matmuls/slice): W Gaussian conv + transpose
    back to h-major.
  evac B (ACT/DVE alternating; GpSimd cannot read PSUM on TRN2):
    PSUM f32 -> SBUF bf16; output DMA'd as bf16 on the GpSimd queue
    (host converts back to f32).
Band matrices encode zero-padding at the edges natively.
"""
import numpy as np
import ml_dtypes

import concourse.bacc as bacc
import concourse.tile as tile
from concourse import mybir
from concourse.bass_utils import run_bass_kernel_spmd

B = 2          # batch
D = 192        # depth
HW = 256       # height = width
SLAB = 48      # output slices per core
DIN = SLAB + 4  # input slices per core (2-slice halo each side)
NB = 130       # band-split matmul N (128 + 2*2 halo)
P = 128
N_CORES = 8

LA = 4         # iterations between pair-adds and pass-A consumption
LB = 3         # iterations between evac-A and pass-B consumption
OCH = 4        # output slices per group DMA
XCHUNKS = [6, 12, 12, 12, 10]  # input slices per chunk DMA (sum = DIN)

F32 = mybir.dt.float32
BF16 = mybir.dt.bfloat16


def _taps():
    c = np.arange(5, dtype=np.float64) - 2
    u = np.exp(-c * c / 2.0)   # D-axis Gaussian (sigma=1)
    v = np.exp(-c * c)         # W-axis Gaussian (sigma^2=1/2)
    aD = (u / u.sum()).astype(np.float64)
    bW = (v / v.sum()).astype(np.float64)
    return aD, bW


def _band(rows, cols, roff, coff, taps):
    """M[r, c] = taps[(r+roff) - (c+coff) + 2] where |diff| <= 2, else 0."""
    m = np.zeros((rows, cols), dtype=np.float32)
    for r in range(rows):
        g = r + roff
        for c in range(cols):
            d = g - (c + coff)
            if -2 <= d <= 2:
                m[r, c] = taps[d + 2]
    return m


def _const_tensor():
    """[P, 8, NB]: pieces (a0*boxH k0, k1, a1*boxH k0, k1, a2*boxH k0, k1,
    bW k0, k1) — one contiguous DMA."""
    aD, bW = _taps()
    box = np.full(5, 0.2)
    pieces = []
    for coef in (aD[2], aD[1], aD[0]):   # a0 center, a1 inner pair, a2 outer
        t = coef * box
        pieces.append(_band(P, NB, 0, 0, t))
        pieces.append(_band(P, NB, P, HW - NB, t))
    pieces.append(_band(P, NB, 0, 0, bW))
    pieces.append(_band(P, NB, P, HW - NB, bW))
    cb = np.stack(pieces).transpose(1, 0, 2)  # [P, 8, NB]
    return np.ascontiguousarray(cb).astype(ml_dtypes.bfloat16)


def _build_nc():
    nc = bacc.Bacc("TRN2", target_bir_lowering=False, debug=False,
                   num_devices=N_CORES)
    # input pre-swizzled on host: x[p, s, hh, w] = slab[s, hh*128+p, w]
    x_d = nc.declare_dram_parameter("x", [P, DIN, 2, HW], BF16, isOutput=False)
    cb_d = nc.declare_dram_parameter("cb", [P, 8, NB], BF16, isOutput=False)
    # output swizzled: out[p, d, hb, w] = slice_d[hb*128+p, w]
    out_d = nc.declare_dram_parameter("out", [P, SLAB, 2, HW], BF16,
                                      isOutput=True)
    add = mybir.AluOpType.add

    with tile.TileContext(nc) as tc:
        with (
            tc.tile_pool(name="consts", bufs=1) as cpool,
            tc.tile_pool(name="xbf", bufs=1) as xpool,
            tc.tile_pool(name="tp", bufs=8) as tpool,
            tc.tile_pool(name="y", bufs=4) as ypool,
            tc.tile_pool(name="osb", bufs=3) as opool,
            tc.tile_pool(name="pa", bufs=2, space="PSUM") as pa_pool,
            tc.tile_pool(name="pb", bufs=2, space="PSUM") as pb_pool,
        ):
            cb_sb = cpool.tile([P, 8, NB], BF16, tag="cb")
            x_sb = xpool.tile([P, DIN, 2, HW], BF16, tag="x")

            # consts first (first pass-A matmul needs them), then x chunks,
            # all on the SP hwdge queue
            nc.sync.dma_start(cb_sb[:], cb_d[:])
            st = 0
            for n in XCHUNKS:
                nc.sync.dma_start(x_sb[:, st:st + n], x_d[:, st:st + n])
                st += n
            assert st == DIN

            def cpiece(i):
                return cb_sb[:, i]

            t1s, t2s, ys2 = [], [], []
            a_ps = None
            o_ps = None
            o_sb = None
            for it in range(SLAB + LA + LB):
                # pair adds for output slices (2q, 2q+1)
                if it % 2 == 0 and it // 2 < SLAB // 2:
                    q = it // 2
                    d0 = 2 * q
                    t1 = tpool.tile([P, 2, 2, HW], BF16, tag="t1")
                    t2 = tpool.tile([P, 2, 2, HW], BF16, tag="t2")
                    t1s.append(t1)
                    t2s.append(t2)
                    nc.vector.tensor_tensor(
                        t1[:], x_sb[:, d0 + 1:d0 + 3], x_sb[:, d0 + 3:d0 + 5],
                        add)
                    if q % 2 == 0:
                        nc.vector.tensor_tensor(
                            t2[:], x_sb[:, d0:d0 + 2], x_sb[:, d0 + 4:d0 + 6],
                            add)
                    else:
                        nc.gpsimd.tensor_add(
                            t2[:], x_sb[:, d0:d0 + 2], x_sb[:, d0 + 4:d0 + 6])

                # pass A: D gauss + H box conv + transpose -> w-major
                da = it - LA
                if 0 <= da < SLAB:
                    if da % 2 == 0:
                        a_ps = pa_pool.tile([P, 2, 2, HW], F32, tag="aps")
                    srcs = (
                        (x_sb[:, da + 2], 0),
                        (t1s[da // 2][:, da % 2], 2),
                        (t2s[da // 2][:, da % 2], 4),
                    )
                    n_mm = 0
                    for src, cbase in srcs:
                        for wblk in range(2):
                            nc.tensor.matmul(
                                a_ps[:, da % 2, wblk, 0:NB],
                                src[:, 0, wblk * P: wblk * P + P],
                                cpiece(cbase),
                                start=n_mm == 0, stop=False)
                            nc.tensor.matmul(
                                a_ps[:, da % 2, wblk, HW - NB:HW],
                                src[:, 1, wblk * P: wblk * P + P],
                                cpiece(cbase + 1),
                                start=False, stop=n_mm == 5)
                            n_mm += 1
                    if da % 2 == 1:
                        y2 = ypool.tile([P, 2, 2, HW], BF16, tag="y")
                        ys2.append(y2)
                        nc.scalar.copy(y2[:], a_ps[:])

                # pass B: W gauss conv + transpose back to h-major
                db = da - LB
                if not (0 <= db < SLAB):
                    continue
                if db % 2 == 0:
                    o_ps = pb_pool.tile([P, 2, 2, HW], F32, tag="ops")
                ysrc = ys2[db // 2][:, db % 2]
                n_mm = 0
                for kh in range(2):
                    rhs = cpiece(6 + kh)
                    col0 = 0 if kh == 0 else HW - NB
                    for hblk in range(2):
                        nc.tensor.matmul(
                            o_ps[:, db % 2, hblk, col0: col0 + NB],
                            ysrc[:, kh, hblk * P: hblk * P + P],
                            rhs,
                            start=n_mm == 0, stop=n_mm == 3)
                        n_mm += 1

                if db % OCH == 0:
                    o_sb = opool.tile([P, OCH, 2, HW], BF16, tag="osb")
                if db % 2 == 1:
                    dst = o_sb[:, db % OCH - 1: db % OCH + 1]
                    if (db // 2) % 2 == 0:
                        nc.scalar.copy(dst, o_ps[:])
                    else:
                        nc.vector.tensor_copy(dst, o_ps[:])
                if db % OCH == OCH - 1:
                    nc.gpsimd.dma_start(
                        out_d[:, db - OCH + 1: db + 1], o_sb[:])

    nc.compile()
    return nc


_NC_CACHE = {}


def _get_nc():
    if "nc" not in _NC_CACHE:
        _NC_CACHE["nc"] = _build_nc()
    return _NC_CACHE["nc"]


def kernel(x, kernel_size, _trace=False, _trace_kwargs=None):
    """x: (2, 1, 192, 256, 256) float32; kernel_size: 5. Returns same shape."""
    assert int(kernel_size) == 5, "kernel hardcodes kernel_size=5"
    x = np.asarray(x)
    assert x.shape == (B, 1, D, HW, HW), x.shape
    in_dtype = x.dtype

    nc = _get_nc()
    cb = _const_tensor()

    xp = np.zeros((B, D + 4, HW, HW), dtype=ml_dtypes.bfloat16)
    xp[:, 2:D + 2] = x[:, 0].astype(ml_dtypes.bfloat16)

    in_maps = []
    for c in range(N_CORES):
        b, j = divmod(c, 4)
        shard = xp[b, j * SLAB: j * SLAB + DIN]  # [52, 256, 256]
        sw = np.ascontiguousarray(
            shard.reshape(DIN, 2, P, HW).transpose(2, 0, 1, 3))
        in_maps.append({
            "x": sw,
            "cb": cb,
        })

    res = run_bass_kernel_spmd(
        nc, in_maps, core_ids=list(range(N_CORES)),
        trace=_trace, **(_trace_kwargs or {}))

    out = np.empty((B, 1, D, HW, HW), dtype=np.float32)
    for c in range(N_CORES):
        b, j = divmod(c, 4)
        r = res.results[c]["out"]  # [128, 48, 2, 256] bf16
        out[b, 0, j * SLAB:(j + 1) * SLAB] = (
            r.astype(np.float32).transpose(1, 2, 0, 3).reshape(SLAB, HW, HW))

    if _trace:
        kernel._last_result = res
    return out.astype(in_dtype, copy=False)


# revision 10
# speedup vs baseline: 1.2700x; 1.2700x over previous
"""3D Gaussian blur (kernel_size=5, sigma=1.0) on (2,1,192,256,256) f32,
distributed over 8 Trainium2 NeuronCores.

The reference kernel factors exactly: g[i,j,l] = aD[i] * (1/5) * bW[l],
so the 3D conv separates into: Gaussian along D, box along H, Gaussian
along W.

Sharding: data-parallel over (batch, D-slab): 8 cores = 2 batches x 4
slabs of 48 output slices each; each core receives its slab plus a
2-slice halo (zero slices at batch edges), i.e. input [52, 256, 256].

Per-core kernel (Bass/Tile), v2 — D-conv via symmetric pairing:
  pairs (DVE / GpSimd alternating): for each output slice dd,
    t1 = x[dd+1]+x[dd+3] (bf16 adds batched 2 output slices per op on
    the contiguous flat x tile). The outer a2 taps stay on the PE (the
    elementwise engines are too slow to pair them too, and the PE must
    remain the clear bottleneck to hold its ramped 2.4 GHz p-state).
  pass A (PE, 16 banded matmuls/slice): y[dd] = sum over the 4 sources
    {x[dd+2]:a0, t1:a1, x[dd]:a2, x[dd+4]:a2} of (aD_v * box_H)-band
    matmuls, fusing the D Gaussian + H box conv with an H<->W
    transpose, PSUM-accumulated.
  evac A (ACT): PSUM f32 -> SBUF bf16 per 2 slices.
  pass B (PE, 4 banded matmuls/slice): W Gaussian conv + transpose
    back to h-major.
  evac B (ACT/DVE alternating; GpSimd cannot read PSUM on TRN2):
    PSUM f32 -> SBUF bf16; output DMA'd as bf16 on the GpSimd queue
    (host converts back to f32).
Band matrices encode zero-padding at the edges natively.
"""
import numpy as np
import ml_dtypes

import concourse.bacc as bacc
import concourse.tile as tile
from concourse import mybir
from concourse.bass_utils import run_bass_kernel_spmd

B = 2          # batch
D = 192        # depth
HW = 256       # height = width
SLAB = 48      # output slices per core
DIN = SLAB + 4  # input slices per core (2-slice halo each side)
NB = 130       # band-split matmul N (128 + 2*2 halo)
P = 128
N_CORES = 8

LA = 4         # iterations between pair-adds and pass-A consumption
LB = 3         # iterations between evac-A and pass-B consumption
OCH = 4        # output slices per group DMA
XCHUNKS = [6, 12, 12, 12, 10]  # input slices per chunk DMA (sum = DIN)

F32 = mybir.dt.float32
BF16 = mybir.dt.bfloat16


def _taps():
    c = np.arange(5, dtype=np.float64) - 2
    u = np.exp(-c * c / 2.0)   # D-axis Gaussian (sigma=1)
    v = np.exp(-c * c)         # W-axis Gaussian (sigma^2=1/2)
    aD = (u / u.sum()).astype(np.float64)
    bW = (v / v.sum()).astype(np.float64)
    return aD, bW


def _band(rows, cols, roff, coff, taps):
    """M[r, c] = taps[(r+roff) - (c+coff) + 2] where |diff| <= 2, else 0."""
    m = np.zeros((rows, cols), dtype=np.float32)
    for r in range(rows):
        g = r + roff
        for c in range(cols):
            d = g - (c + coff)
            if -2 <= d <= 2:
                m[r, c] = taps[d + 2]
    return m


def _const_tensor():
    """[P, 8, NB]: pieces (a0*boxH k0, k1, a1*boxH k0, k1, a2*boxH k0, k1,
    bW k0, k1) — one contiguous DMA."""
    aD, bW = _taps()
    box = np.full(5, 0.2)
    pieces = []
    for coef in (aD[2], aD[1], aD[0]):   # a0 center, a1 inner pair, a2 outer
        t = coef * box
        pieces.append(_band(P, NB, 0, 0, t))
        pieces.append(_band(P, NB, P, HW - NB, t))
    pieces.append(_band(P, NB, 0, 0, bW))
    pieces.append(_band(P, NB, P, HW - NB, bW))
    cb = np.stack(pieces).transpose(1, 0, 2)  # [P, 8, NB]
    return np.ascontiguousarray(cb).astype(ml_dtypes.bfloat16)


def _build_nc():
    nc = bacc.Bacc("TRN2", target_bir_lowering=False, debug=False,
                   num_devices=N_CORES)
    # input pre-swizzled on host: x[p, s, hh, w] = slab[s, hh*128+p, w]
    x_d = nc.declare_dram_parameter("x", [P, DIN, 2, HW], BF16, isOutput=False)
    cb_d = nc.declare_dram_parameter("cb", [P, 8, NB], BF16, isOutput=False)
    # output swizzled: out[p, d, hb, w] = slice_d[hb*128+p, w]
    out_d = nc.declare_dram_parameter("out", [P, SLAB, 2, HW], BF16,
                                      isOutput=True)
    add = mybir.AluOpType.add

    with tile.TileContext(nc) as tc:
        with (
            tc.tile_pool(name="consts", bufs=1) as cpool,
            tc.tile_pool(name="xbf", bufs=1) as xpool,
            tc.tile_pool(name="tp", bufs=8) as tpool,
            tc.tile_pool(name="y", bufs=4) as ypool,
            tc.tile_pool(name="osb", bufs=3) as opool,
            tc.tile_pool(name="pa", bufs=2, space="PSUM") as pa_pool,
            tc.tile_pool(name="pb", bufs=2, space="PSUM") as pb_pool,
        ):
            cb_sb = cpool.tile([P, 8, NB], BF16, tag="cb")
            x_sb = xpool.tile([P, DIN, 2, HW], BF16, tag="x")

            # consts first (first pass-A matmul needs them), then x chunks,
            # all on the SP hwdge queue
            nc.sync.dma_start(cb_sb[:], cb_d[:])
            st = 0
            for n in XCHUNKS:
                nc.sync.dma_start(x_sb[:, st:st + n], x_d[:, st:st + n])
                st += n
            assert st == DIN

            def cpiece(i):
                return cb_sb[:, i]

            # PE p-state warmup: keep the tensor engine busy through the
            # input-DMA wait so the real matmul stream starts fully ramped
            # (TRN2 needs ~3us of continuous PE activity for max clock).
            scr = cpool.tile([P, 512], BF16, tag="scr")
            nc.vector.memset(scr[:], 0.0)
            wu_ps = pb_pool.tile([P, 2, 2, HW], F32, tag="ops")
            for _ in range(32):
                nc.tensor.matmul(wu_ps[:, 0], scr[:, 0:128], scr[:],
                                 start=True, stop=True)

            t1s, ys2 = [], []
            a_ps = None
            o_ps = None
            o_sb = None
            for it in range(SLAB + LA + LB):
                # pair add for output slices (2q, 2q+1)
                if it % 2 == 0 and it // 2 < SLAB // 2:
                    q = it // 2
                    d0 = 2 * q
                    t1 = tpool.tile([P, 2, 2, HW], BF16, tag="t1")
                    t1s.append(t1)
                    if q % 2 == 0:
                        nc.vector.tensor_tensor(
                            t1[:], x_sb[:, d0 + 1:d0 + 3],
                            x_sb[:, d0 + 3:d0 + 5], add)
                    else:
                        nc.gpsimd.tensor_add(
                            t1[:], x_sb[:, d0 + 1:d0 + 3],
                            x_sb[:, d0 + 3:d0 + 5])

                # pass A: D gauss + H box conv + transpose -> w-major
                da = it - LA
                if 0 <= da < SLAB:
                    if da % 2 == 0:
                        a_ps = pa_pool.tile([P, 2, 2, HW], F32, tag="aps")
                    srcs = (
                        (x_sb[:, da + 2], 0),
                        (t1s[da // 2][:, da % 2], 2),
                        (x_sb[:, da], 4),
                        (x_sb[:, da + 4], 4),
                    )
                    n_mm = 0
                    for src, cbase in srcs:
                        for wblk in range(2):
                            nc.tensor.matmul(
                                a_ps[:, da % 2, wblk, 0:NB],
                                src[:, 0, wblk * P: wblk * P + P],
                                cpiece(cbase),
                                start=n_mm == 0, stop=False)
                            nc.tensor.matmul(
                                a_ps[:, da % 2, wblk, HW - NB:HW],
                                src[:, 1, wblk * P: wblk * P + P],
                                cpiece(cbase + 1),
                                start=False, stop=n_mm == 7)
                            n_mm += 1
                    if da % 2 == 1:
                        y2 = ypool.tile([P, 2, 2, HW], BF16, tag="y")
                        ys2.append(y2)
                        nc.scalar.copy(y2[:], a_ps[:])

                # pass B: W gauss conv + transpose back to h-major
                db = da - LB
                if not (0 <= db < SLAB):
                    continue
                if db % 2 == 0:
                    o_ps = pb_pool.tile([P, 2, 2, HW], F32, tag="ops")
                ysrc = ys2[db // 2][:, db % 2]
                n_mm = 0
                for kh in range(2):
                    rhs = cpiece(6 + kh)
                    col0 = 0 if kh == 0 else HW - NB
                    for hblk in range(2):
                        nc.tensor.matmul(
                            o_ps[:, db % 2, hblk, col0: col0 + NB],
                            ysrc[:, kh, hblk * P: hblk * P + P],
                            rhs,
                            start=n_mm == 0, stop=n_mm == 3)
                        n_mm += 1

                if db % OCH == 0:
                    o_sb = opool.tile([P, OCH, 2, HW], BF16, tag="osb")
                if db % 2 == 1:
                    dst = o_sb[:, db % OCH - 1: db % OCH + 1]
                    if (db // 2) % 2 == 0:
                        nc.scalar.copy(dst, o_ps[:])
                    else:
                        nc.vector.tensor_copy(dst, o_ps[:])
                if db % OCH == OCH - 1:
                    nc.gpsimd.dma_start(
                        out_d[:, db - OCH + 1: db + 1], o_sb[:])

    nc.compile()
    return nc


_NC_CACHE = {}


def _get_nc():
    if "nc" not in _NC_CACHE:
        _NC_CACHE["nc"] = _build_nc()
    return _NC_CACHE["nc"]


def kernel(x, kernel_size, _trace=False, _trace_kwargs=None):
    """x: (2, 1, 192, 256, 256) float32; kernel_size: 5. Returns same shape."""
    assert int(kernel_size) == 5, "kernel hardcodes kernel_size=5"
    x = np.asarray(x)
    assert x.shape == (B, 1, D, HW, HW), x.shape
    in_dtype = x.dtype

    nc = _get_nc()
    cb = _const_tensor()

    xp = np.zeros((B, D + 4, HW, HW), dtype=ml_dtypes.bfloat16)
    xp[:, 2:D + 2] = x[:, 0].astype(ml_dtypes.bfloat16)

    in_maps = []
    for c in range(N_CORES):
        b, j = divmod(c, 4)
        shard = xp[b, j * SLAB: j * SLAB + DIN]  # [52, 256, 256]
        sw = np.ascontiguousarray(
            shard.reshape(DIN, 2, P, HW).transpose(2, 0, 1, 3))
        in_maps.append({
            "x": sw,
            "cb": cb,
        })

    res = run_bass_kernel_spmd(
        nc, in_maps, core_ids=list(range(N_CORES)),
        trace=_trace, **(_trace_kwargs or {}))

    out = np.empty((B, 1, D, HW, HW), dtype=np.float32)
    for c in range(N_CORES):
        b, j = divmod(c, 4)
        r = res.results[c]["out"]  # [128, 48, 2, 256] bf16
        out[b, 0, j * SLAB:(j + 1) * SLAB] = (
            r.astype(np.float32).transpose(1, 2, 0, 3).reshape(SLAB, HW, HW))

    if _trace:
        kernel._last_result = res
    return out.astype(in_dtype, copy=False)
